# revision 1
# baseline (speedup 1.0000x reference)
"""Trainium2 Bass kernel for nn_LoRAPool (MoE top-2 LoRA expert pool).

Math (reference):
    gates[t,e] = p_L[t,e] if e in top-2 of p_L[t,:] else 0
    hr[t,e,r]  = sum_d h[t,d] * A[e,r,d]
    out[t,d]   = sum_{e,r} hr[t,e,r] * 2.0 * gates[t,e] * B[e,d,r]

Folded into two dense matmuls over c = (e,r) in [0,128):
    A_cat[d,c] = 2.0 * A[e,r,d];  B_cat[c,d] = B[e,d,r]
    U^T[c,t]   = sum_d A_cat[d,c] h[t,d]        (stage 1, PE)
    Us[c,t]    = U^T[c,t] * gates[t, c//16]     (gating, DVE)
    out[t,d]   = sum_c Us[c,t] B_cat[c,d]       (stage 2, PE)

Sharding: tokens (4*4096 = 16384) split evenly across 8 cores; A/B and
small helper matrices are replicated.
"""

import numpy as np

N_CORES = 8
B_SZ, S_SZ, D = 4, 4096, 2048
E, R, C = 8, 16, 128
T_FULL = B_SZ * S_SZ            # 16384 tokens
T_CORE = T_FULL // N_CORES      # 2048 tokens per core
GROUP = 512                     # token group (matmul moving dim)
N_GROUPS = T_CORE // GROUP      # 4
N_SUB = GROUP // 128            # 4 sub-tiles of 128 tokens
KD = D // 128                   # 16 contraction chunks
SCALING = 2.0

_CACHE = {}


def _build_nc(use_f32r=True, split_waits=True):
    import concourse.bass as bass
    import concourse.tile as tile
    import concourse.mybir as mybir
    from contextlib import ExitStack

    f32 = mybir.dt.float32
    mm_dt = mybir.dt.float32r if use_f32r else f32

    nc = bass.Bass()
    h_d = nc.declare_dram_parameter("h", [T_CORE, D], f32, isOutput=False)
    p_d = nc.declare_dram_parameter("p_L", [T_CORE, E], f32, isOutput=False)
    a_d = nc.declare_dram_parameter("A_cat", [D, C], f32, isOutput=False)
    b_d = nc.declare_dram_parameter("B_cat", [C, D], f32, isOutput=False)
    m_d = nc.declare_dram_parameter("Mexp", [E, C], f32, isOutput=False)
    i_d = nc.declare_dram_parameter("Ident", [128, 128], f32, isOutput=False)
    o_d = nc.declare_dram_parameter("out", [T_CORE, D], f32, isOutput=True)

    AX = mybir.AxisListType
    OP = mybir.AluOpType

    with ExitStack() as ctx:
        tc = ctx.enter_context(tile.TileContext(nc))
        consts = ctx.enter_context(tc.tile_pool(name="consts", bufs=1))
        hpool = ctx.enter_context(tc.tile_pool(name="h", bufs=2 * N_SUB))
        htpool = ctx.enter_context(tc.tile_pool(name="hT", bufs=4))
        utspool = ctx.enter_context(tc.tile_pool(name="uts", bufs=2))
        outpool = ctx.enter_context(tc.tile_pool(name="osb", bufs=3))
        gpool = ctx.enter_context(tc.tile_pool(name="gates", bufs=2))
        ps_ht = ctx.enter_context(tc.tile_pool(name="ps_ht", bufs=2, space="PSUM"))
        # gT, G, U rotate through one 3-slot pool (1 bank each)
        ps_acc = ctx.enter_context(tc.tile_pool(name="ps_acc", bufs=3, space="PSUM"))
        ps_out = ctx.enter_context(tc.tile_pool(name="ps_out", bufs=3, space="PSUM"))

        A_raw = consts.tile([128, KD, C], f32)
        nc.sync.dma_start(out=A_raw, in_=a_d.rearrange("(k p) c -> p k c", p=128))
        A_sb = consts.tile([128, KD, C], mm_dt)
        nc.vector.tensor_copy(out=A_sb, in_=A_raw)
        B_raw = consts.tile([C, D], f32)
        nc.sync.dma_start(out=B_raw, in_=b_d[:, :])
        B_sb = consts.tile([C, D], mm_dt)
        nc.vector.tensor_copy(out=B_sb, in_=B_raw)
        M_sb = consts.tile([E, C], f32)
        nc.sync.dma_start(out=M_sb, in_=m_d[:, :])
        I_sb = consts.tile([128, 128], f32)
        nc.sync.dma_start(out=I_sb, in_=i_d[:, :])

        for g in range(N_GROUPS):
            t0 = g * GROUP

            h_tiles = []
            for s in range(N_SUB):
                ht = hpool.tile([128, D], f32, tag="h")
                nc.sync.dma_start(
                    out=ht, in_=h_d[t0 + s * 128 : t0 + (s + 1) * 128, :]
                )
                h_tiles.append(ht)

            # ---- top-2 gates on [128 tokens, N_SUB, E] ----
            p_sb = gpool.tile([128, N_SUB, E], f32, tag="p")
            nc.sync.dma_start(
                out=p_sb,
                in_=p_d[t0 : t0 + GROUP, :].rearrange("(s p) e -> p s e", p=128),
            )
            m1 = gpool.tile([128, N_SUB, 1], f32, tag="m1")
            nc.vector.tensor_reduce(out=m1, in_=p_sb, axis=AX.X, op=OP.max)
            mlt = gpool.tile([128, N_SUB, E], f32, tag="mlt")
            nc.vector.tensor_tensor(
                out=mlt, in0=p_sb, in1=m1.broadcast_to([128, N_SUB, E]), op=OP.is_lt
            )
            pm = gpool.tile([128, N_SUB, E], f32, tag="pm")
            nc.vector.tensor_mul(pm, p_sb, mlt)
            m2 = gpool.tile([128, N_SUB, 1], f32, tag="m2")
            nc.vector.tensor_reduce(out=m2, in_=pm, axis=AX.X, op=OP.max)
            ge2 = gpool.tile([128, N_SUB, E], f32, tag="ge2")
            nc.vector.tensor_tensor(
                out=ge2, in0=p_sb, in1=m2.broadcast_to([128, N_SUB, E]), op=OP.is_ge
            )
            gts = gpool.tile([128, N_SUB, E], f32, tag="gts")
            nc.vector.tensor_mul(gts, p_sb, ge2)

            # transpose gates -> gT[e, t] and expand to G[c, t] via one-hot matmul
            gt_ps = ps_acc.tile([128, GROUP], f32, tag="acc")
            for s in range(N_SUB):
                nc.tensor.transpose(
                    out=gt_ps[:E, s * 128 : (s + 1) * 128],
                    in_=gts[:, s, :],
                    identity=I_sb,
                )
            gt_sb = gpool.tile([E, GROUP], f32, tag="gtsb")
            nc.vector.tensor_copy(out=gt_sb, in_=gt_ps[:E, :])
            G_ps = ps_acc.tile([128, GROUP], f32, tag="acc")
            nc.tensor.matmul(G_ps, lhsT=M_sb, rhs=gt_sb, start=True, stop=True)
            G_sb = gpool.tile([128, GROUP], f32, tag="gsb")
            nc.vector.tensor_copy(out=G_sb, in_=G_ps)

            # ---- stage 1: U^T[c, t] accumulated over 16 d-chunks ----
            U_ps = ps_acc.tile([128, GROUP], f32, tag="acc")
            for k in range(KD):
                ht_ps = ps_ht.tile([128, GROUP], f32, tag="htp")
                for s in range(N_SUB):
                    nc.tensor.transpose(
                        out=ht_ps[:, s * 128 : (s + 1) * 128],
                        in_=h_tiles[s][:, k * 128 : (k + 1) * 128],
                        identity=I_sb,
                    )
                ht_sb = htpool.tile([128, GROUP], mm_dt, tag="hts")
                # DVE copies ~1.6x faster than ACT: split 10/6 per group
                if k % 8 < 5:
                    nc.vector.tensor_copy(out=ht_sb, in_=ht_ps)
                else:
                    nc.scalar.copy(out=ht_sb, in_=ht_ps)
                nc.tensor.matmul(
                    U_ps,
                    lhsT=A_sb[:, k, :],
                    rhs=ht_sb[:, :],
                    start=(k == 0),
                    stop=(k == KD - 1),
                )

            # ---- gating ----
            uts = utspool.tile([128, GROUP], mm_dt, tag="uts")
            nc.vector.tensor_tensor(out=uts, in0=U_ps, in1=G_sb, op=OP.mult)

            # ---- stage 2: out[t, d] per 128-token sub-tile ----
            for s in range(N_SUB):
                o_sb = outpool.tile([128, D], f32, tag="osb")
                for j in range(D // 512):
                    o_ps = ps_out.tile([128, 512], f32, tag="ops")
                    nc.tensor.matmul(
                        o_ps,
                        lhsT=uts[:, s * 128 : (s + 1) * 128],
                        rhs=B_sb[:, j * 512 : (j + 1) * 512],
                        start=True,
                        stop=True,
                    )
                    if (s * 4 + j) % 16 < 9:
                        nc.vector.tensor_copy(
                            out=o_sb[:, j * 512 : (j + 1) * 512], in_=o_ps
                        )
                    else:
                        nc.scalar.copy(out=o_sb[:, j * 512 : (j + 1) * 512], in_=o_ps)
                nc.sync.dma_start(
                    out=o_d[t0 + s * 128 : t0 + (s + 1) * 128, :], in_=o_sb
                )

    if split_waits:
        _split_matmul_waits(nc)
    return nc


def _split_matmul_waits(nc, max_waits=1):
    """Walrus codegen allows only one sync-wait slot on self-loading
    (fp32/fp32r) Matmult instructions. Move surplus waits onto a no-op
    EventSemaphore inserted immediately before, same engine — identical
    semantics (waits still complete before the matmul dispatches)."""
    import concourse.mybir as mybir

    n = 0
    for f in nc.m.functions:
        for blk in f.blocks:
            insts = blk.instructions
            new_list = []
            changed = False
            for inst in insts:
                si = inst.sync_info
                if (
                    type(inst).__name__ != "InstEventSemaphore"
                    and si is not None
                    and si.on_wait
                    and len(si.on_wait) > max_waits
                ):
                    surplus = list(si.on_wait[:-max_waits])
                    keep = list(si.on_wait[-max_waits:])
                    # EventSemaphore carriers take at most 2 waits each
                    for i in range(0, len(surplus), 2):
                        n += 1
                        ev = mybir.InstEventSemaphore(
                            name=f"I-swsplit-{n}", ins=[], outs=[]
                        )
                        ev.engine = inst.engine
                        ev.sync_info = mybir.SyncInfo(
                            on_wait=surplus[i : i + 2], on_update=[]
                        )
                        new_list.append(ev)
                    inst.sync_info = mybir.SyncInfo(
                        on_wait=keep, on_update=list(si.on_update or [])
                    )
                    changed = True
                new_list.append(inst)
            if changed:
                blk.instructions = new_list
    return n


def _host_prep(h, p_L, A, B):
    """Shard tokens across cores; build replicated helper matrices."""
    h_flat = np.ascontiguousarray(h.reshape(T_FULL, D), dtype=np.float32)
    p_flat = np.ascontiguousarray(p_L.reshape(T_FULL, E), dtype=np.float32)
    # A_cat[d, c] = SCALING * A[e, r, d]
    A_cat = np.ascontiguousarray(
        (np.asarray(A, dtype=np.float32) * SCALING).transpose(2, 0, 1).reshape(D, C)
    )
    # B_cat[c, d] = B[e, d, r]
    B_cat = np.ascontiguousarray(
        np.asarray(B, dtype=np.float32).transpose(0, 2, 1).reshape(C, D)
    )
    Mexp = np.zeros((E, C), dtype=np.float32)
    for e in range(E):
        Mexp[e, e * R : (e + 1) * R] = 1.0
    Ident = np.eye(128, dtype=np.float32)
    in_maps = []
    for i in range(N_CORES):
        sl = slice(i * T_CORE, (i + 1) * T_CORE)
        in_maps.append(
            {
                "h": h_flat[sl],
                "p_L": p_flat[sl],
                "A_cat": A_cat,
                "B_cat": B_cat,
                "Mexp": Mexp,
                "Ident": Ident,
            }
        )
    return in_maps


def _get_nc():
    if "nc" not in _CACHE:
        _CACHE["nc"] = _build_nc()
    return _CACHE["nc"]


def kernel(h, p_L, A, B):
    from concourse.bass_utils import run_bass_kernel_spmd

    nc = _get_nc()
    in_maps = _host_prep(h, p_L, A, B)
    res = run_bass_kernel_spmd(nc, in_maps, core_ids=list(range(N_CORES)))
    out = np.concatenate([res.results[i]["out"] for i in range(N_CORES)], axis=0)
    return out.reshape(B_SZ, S_SZ, D)



# revision 5
# speedup vs baseline: 1.4175x; 1.4175x over previous
"""Trainium2 Bass kernel for nn_LoRAPool (MoE top-2 LoRA expert pool).

Math (reference):
    gates[t,e] = p_L[t,e] if e in top-2 of p_L[t,:] else 0
    hr[t,e,r]  = sum_d h[t,d] * A[e,r,d]
    out[t,d]   = sum_{e,r} hr[t,e,r] * 2.0 * gates[t,e] * B[e,d,r]

Folded into two dense matmuls over c = (e,r) in [0,128):
    A_cat[d,c] = 2.0 * A[e,r,d];  B_cat[c,d] = B[e,d,r]
    U^T[c,t]   = sum_d A_cat[d,c] h[t,d]        (stage 1, PE)
    Us[c,t]    = U^T[c,t] * gates[t, c//16]     (gating, DVE)
    out[t,d]   = sum_c Us[c,t] B_cat[c,d]       (stage 2, PE)

Key layout trick: h is fed to the device in bf16 and transposed during
the HBM->SBUF DMA by the XBAR transpose engine (InstDmaTransposeAnt),
so the PE never spends cycles transposing h. All matmuls run in bf16
(1 cycle/row); top-2 selection stays in f32 for exact expert choice.
I/O is bf16 (h in, out out) which halves HBM traffic vs f32.

Sharding: tokens (4*4096 = 16384) split evenly across 8 cores; A/B and
the small expert-expand matrix are replicated.
"""

import numpy as np
import ml_dtypes

N_CORES = 8
B_SZ, S_SZ, D = 4, 4096, 2048
E, R, C = 8, 16, 128
T_FULL = B_SZ * S_SZ            # 16384 tokens
T_CORE = T_FULL // N_CORES      # 2048 tokens per core
GROUP = 512                     # token group (matmul moving dim)
N_GROUPS = T_CORE // GROUP      # 4
N_SUB = GROUP // 128            # 4 sub-tiles of 128 tokens
KD = D // 128                   # 16 contraction chunks
SCALING = 2.0

_CACHE = {}


def _build_nc(split_waits=True):
    import concourse.bass as bass
    import concourse.tile as tile
    import concourse.mybir as mybir
    from contextlib import ExitStack

    f32 = mybir.dt.float32
    f32r = mybir.dt.float32r
    bf16 = mybir.dt.bfloat16

    nc = bass.Bass()
    h_d = nc.declare_dram_parameter("h", [T_CORE, D], bf16, isOutput=False)
    p_d = nc.declare_dram_parameter("p_L", [T_CORE, E], f32, isOutput=False)
    a_d = nc.declare_dram_parameter("A3", [128, KD, C], bf16, isOutput=False)
    b_d = nc.declare_dram_parameter("B_cat", [C, D], bf16, isOutput=False)
    m_d = nc.declare_dram_parameter("Mexp", [E, C], f32r, isOutput=False)
    i_d = nc.declare_dram_parameter("Ident", [128, 128], f32, isOutput=False)
    o_d = nc.declare_dram_parameter("out", [T_CORE, D], bf16, isOutput=True)

    AX = mybir.AxisListType
    OP = mybir.AluOpType

    with ExitStack() as ctx:
        tc = ctx.enter_context(tile.TileContext(nc))
        consts = ctx.enter_context(tc.tile_pool(name="consts", bufs=1))
        htpool = ctx.enter_context(tc.tile_pool(name="ht", bufs=2))
        gpool = ctx.enter_context(tc.tile_pool(name="gates", bufs=2))
        utspool = ctx.enter_context(tc.tile_pool(name="uts", bufs=2))
        outpool = ctx.enter_context(tc.tile_pool(name="osb", bufs=3))
        ps_u = ctx.enter_context(tc.tile_pool(name="ps_u", bufs=2, space="PSUM"))
        ps_g = ctx.enter_context(tc.tile_pool(name="ps_g", bufs=2, space="PSUM"))
        ps_o = ctx.enter_context(tc.tile_pool(name="ps_o", bufs=2, space="PSUM"))

        A_sb = consts.tile([128, KD, C], bf16)
        nc.sync.dma_start(out=A_sb, in_=a_d[:, :, :])
        B_sb = consts.tile([C, D], bf16)
        nc.sync.dma_start(out=B_sb, in_=b_d[:, :])
        M_sb = consts.tile([E, C], f32r)
        nc.sync.dma_start(out=M_sb, in_=m_d[:, :])
        I_sb = consts.tile([128, 128], f32)
        nc.sync.dma_start(out=I_sb, in_=i_d[:, :])

        for g in range(N_GROUPS):
            t0 = g * GROUP

            # h^T via XBAR transpose DMA: ht[c, k, t] = h[t0+t, 128k+c]
            ht = htpool.tile([128, KD, GROUP], bf16, tag="ht")
            nc.sync.dma_start_transpose(out=ht, in_=h_d[t0 : t0 + GROUP, :])

            # ---- top-2 gates on [128 tokens, N_SUB, E] (f32, exact) ----
            p_sb = gpool.tile([128, N_SUB, E], f32, tag="p")
            nc.sync.dma_start(
                out=p_sb,
                in_=p_d[t0 : t0 + GROUP, :].rearrange("(s p) e -> p s e", p=128),
            )
            m1 = gpool.tile([128, N_SUB, 1], f32, tag="m1")
            nc.vector.tensor_reduce(out=m1, in_=p_sb, axis=AX.X, op=OP.max)
            mlt = gpool.tile([128, N_SUB, E], f32, tag="mlt")
            nc.vector.tensor_tensor(
                out=mlt, in0=p_sb, in1=m1.broadcast_to([128, N_SUB, E]), op=OP.is_lt
            )
            pm = gpool.tile([128, N_SUB, E], f32, tag="pm")
            nc.vector.tensor_mul(pm, p_sb, mlt)
            m2 = gpool.tile([128, N_SUB, 1], f32, tag="m2")
            nc.vector.tensor_reduce(out=m2, in_=pm, axis=AX.X, op=OP.max)
            ge2 = gpool.tile([128, N_SUB, E], f32, tag="ge2")
            nc.vector.tensor_tensor(
                out=ge2, in0=p_sb, in1=m2.broadcast_to([128, N_SUB, E]), op=OP.is_ge
            )
            gts = gpool.tile([128, N_SUB, E], f32, tag="gts")
            nc.vector.tensor_mul(gts, p_sb, ge2)

            # transpose gates -> gT[e, t], expand to G[c, t] via one-hot matmul
            gt_ps = ps_g.tile([128, GROUP], f32, tag="g")
            for s in range(N_SUB):
                nc.tensor.transpose(
                    out=gt_ps[:E, s * 128 : (s + 1) * 128],
                    in_=gts[:, s, :],
                    identity=I_sb,
                )
            gt_sb = gpool.tile([E, GROUP], f32r, tag="gtsb")
            nc.scalar.copy(out=gt_sb, in_=gt_ps[:E, :])
            G_ps = ps_g.tile([128, GROUP], f32, tag="g")
            nc.tensor.matmul(G_ps, lhsT=M_sb, rhs=gt_sb, start=True, stop=True)
            G_sb = gpool.tile([128, GROUP], f32, tag="gsb")
            nc.scalar.copy(out=G_sb, in_=G_ps)

            # ---- stage 1: U^T[c, t] accumulated over 16 d-chunks ----
            U_ps = ps_u.tile([128, GROUP], f32, tag="u")
            for k in range(KD):
                nc.tensor.matmul(
                    U_ps,
                    lhsT=A_sb[:, k, :],
                    rhs=ht[:, k, :],
                    start=(k == 0),
                    stop=(k == KD - 1),
                )

            # ---- gating ----
            uts = utspool.tile([128, GROUP], bf16, tag="uts")
            nc.vector.tensor_tensor(out=uts, in0=U_ps, in1=G_sb, op=OP.mult)

            # ---- stage 2: out[t, d] per 128-token sub-tile ----
            for s in range(N_SUB):
                o_sb = outpool.tile([128, D], bf16, tag="osb")
                for hhalf in range(2):
                    o_ps = ps_o.tile([128, 1024], f32, tag="ops")
                    for j in range(2):
                        d0 = hhalf * 1024 + j * 512
                        nc.tensor.matmul(
                            o_ps[:, j * 512 : (j + 1) * 512],
                            lhsT=uts[:, s * 128 : (s + 1) * 128],
                            rhs=B_sb[:, d0 : d0 + 512],
                            start=True,
                            stop=True,
                        )
                    if (s * 2 + hhalf) % 8 < 3:
                        nc.vector.tensor_copy(
                            out=o_sb[:, hhalf * 1024 : (hhalf + 1) * 1024], in_=o_ps
                        )
                    else:
                        nc.scalar.copy(
                            out=o_sb[:, hhalf * 1024 : (hhalf + 1) * 1024], in_=o_ps
                        )
                nc.sync.dma_start(
                    out=o_d[t0 + s * 128 : t0 + (s + 1) * 128, :], in_=o_sb
                )

    if split_waits:
        _split_matmul_waits(nc)
    return nc


def _split_matmul_waits(nc, max_waits=1):
    """Walrus codegen allows only one sync-wait slot on self-loading
    (fp32/fp32r) Matmult instructions. Move surplus waits onto a no-op
    EventSemaphore inserted immediately before, same engine — identical
    semantics (waits still complete before the matmul dispatches)."""
    import concourse.mybir as mybir

    n = 0
    for f in nc.m.functions:
        for blk in f.blocks:
            insts = blk.instructions
            new_list = []
            changed = False
            for inst in insts:
                si = inst.sync_info
                if (
                    type(inst).__name__ != "InstEventSemaphore"
                    and si is not None
                    and si.on_wait
                    and len(si.on_wait) > max_waits
                ):
                    surplus = list(si.on_wait[:-max_waits])
                    keep = list(si.on_wait[-max_waits:])
                    # EventSemaphore carriers take at most 2 waits each
                    for i in range(0, len(surplus), 2):
                        n += 1
                        ev = mybir.InstEventSemaphore(
                            name=f"I-swsplit-{n}", ins=[], outs=[]
                        )
                        ev.engine = inst.engine
                        ev.sync_info = mybir.SyncInfo(
                            on_wait=surplus[i : i + 2], on_update=[]
                        )
                        new_list.append(ev)
                    inst.sync_info = mybir.SyncInfo(
                        on_wait=keep, on_update=list(si.on_update or [])
                    )
                    changed = True
                new_list.append(inst)
            if changed:
                blk.instructions = new_list
    return n


def _host_prep(h, p_L, A, B):
    """Shard tokens across cores; build replicated helper matrices."""
    bf16 = ml_dtypes.bfloat16
    h_flat = np.ascontiguousarray(
        np.asarray(h, dtype=np.float32).reshape(T_FULL, D)
    ).astype(bf16)
    p_flat = np.ascontiguousarray(np.asarray(p_L, dtype=np.float32).reshape(T_FULL, E))
    # A_cat[d, c] = SCALING * A[e, r, d]
    A_cat = (np.asarray(A, dtype=np.float32) * SCALING).transpose(2, 0, 1).reshape(D, C)
    # XBAR layout: ht[c, k, t] = h[t, 128k + c]  =>  A3[c, k, :] = A_cat[128k+c, :]
    A3 = np.ascontiguousarray(
        A_cat.reshape(KD, 128, C).transpose(1, 0, 2)
    ).astype(bf16)
    # B_cat[c, d] = B[e, d, r]
    B_cat = np.ascontiguousarray(
        np.asarray(B, dtype=np.float32).transpose(0, 2, 1).reshape(C, D)
    ).astype(bf16)
    Mexp = np.zeros((E, C), dtype=np.float32)
    for e in range(E):
        Mexp[e, e * R : (e + 1) * R] = 1.0
    Ident = np.eye(128, dtype=np.float32)
    in_maps = []
    for i in range(N_CORES):
        sl = slice(i * T_CORE, (i + 1) * T_CORE)
        in_maps.append(
            {
                "h": h_flat[sl],
                "p_L": p_flat[sl],
                "A3": A3,
                "B_cat": B_cat,
                "Mexp": Mexp,
                "Ident": Ident,
            }
        )
    return in_maps


def _get_nc():
    if "nc" not in _CACHE:
        _CACHE["nc"] = _build_nc()
    return _CACHE["nc"]


def kernel(h, p_L, A, B):
    from concourse.bass_utils import run_bass_kernel_spmd

    nc = _get_nc()
    in_maps = _host_prep(h, p_L, A, B)
    res = run_bass_kernel_spmd(nc, in_maps, core_ids=list(range(N_CORES)))
    out = np.concatenate(
        [np.asarray(res.results[i]["out"]) for i in range(N_CORES)], axis=0
    )
    return out.astype(np.float32).reshape(B_SZ, S_SZ, D)
